# revision 16
# baseline (speedup 1.0000x reference)
"""DMoNPooling Trainium2 kernel.

B=8 batch elements, data-parallel one per NeuronCore (8 cores).
Per core: x [2048,256], adj [2048,2048], mask [2048], W [256,64], b [64].

Outputs per core: s [2048,64], out [64,256], out_adj [64,64],
cluster_size [64], scalars [spectral_b, ortho_b, csq, count].
Host combines the per-core scalars into the three mean losses,
replicating the reference's f32 arithmetic for the cluster loss.
"""

import numpy as np
from contextlib import ExitStack

import concourse.bass as bass
import concourse.tile as tile
from concourse import bacc
from concourse import mybir
from concourse.masks import make_identity

N, F, C, P = 2048, 256, 64, 128
NCH = N // P          # 16 row chunks
FCH = F // P          # 2 feature blocks
PANW = 512            # adj panel width (columns)
NPAN = N // PANW      # 4 panels
PBLK = PANW // P      # 4 col-blocks per panel

SELU_SCALE = 1.0507009873554805
SELU_ALPHA = 1.6732632423543772
EPS = 1e-15

AF = mybir.ActivationFunctionType
ALU = mybir.AluOpType
AX = mybir.AxisListType
f32 = mybir.dt.float32


def build_program(loop_n=None):
    nc = bacc.Bacc()

    x_in = nc.dram_tensor("x", [N, F], f32, kind="ExternalInput")
    adj_in = nc.dram_tensor("adj", [N, N], f32, kind="ExternalInput")
    mk_in = nc.dram_tensor("maskf", [N], f32, kind="ExternalInput")
    W_in = nc.dram_tensor("w", [F, C], f32, kind="ExternalInput")
    b_in = nc.dram_tensor("bvec", [C], f32, kind="ExternalInput")

    s_out = nc.dram_tensor("s_out", [N, C], f32, kind="ExternalOutput")
    o_out = nc.dram_tensor("o_out", [C, F], f32, kind="ExternalOutput")
    oa_out = nc.dram_tensor("oa_out", [C, C], f32, kind="ExternalOutput")
    sc_out = nc.dram_tensor("sc_out", [4], f32, kind="ExternalOutput")
    cs_out = nc.dram_tensor("cs_out", [C], f32, kind="ExternalOutput")

    adj_r = adj_in.rearrange("(o p) n -> p o n", p=P)
    x_r = x_in.rearrange("(o p) f -> p o f", p=P)
    s_r = s_out.rearrange("(o p) c -> p o c", p=P)

    with tile.TileContext(nc) as tc, ExitStack() as ctx:
        singles = ctx.enter_context(tc.tile_pool(name="singles", bufs=1))

        # --- constants (outside any timing loop) ---
        W_sb = singles.tile([P, FCH, C], f32)
        nc.sync.dma_start(out=W_sb, in_=W_in.rearrange("(f p) c -> p f c", p=P))
        b_bc = singles.tile([P, C], f32)
        nc.gpsimd.dma_start(out=b_bc, in_=b_in[:].partition_broadcast(P))
        mask_sb = singles.tile([P, NCH], f32)
        nc.sync.dma_start(out=mask_sb, in_=mk_in.rearrange("(o p) -> p o", p=P))
        ident = singles.tile([P, P], f32)
        make_identity(nc, ident)
        ones_col = singles.tile([C, 1], f32)
        nc.vector.memset(ones_col, 1.0)
        invI = singles.tile([C, C], f32)
        nc.vector.memset(invI, 1.0)
        nc.gpsimd.affine_select(
            out=invI, in_=invI, compare_op=ALU.not_equal, fill=0.0,
            base=0, pattern=[[-1, C]], channel_multiplier=1,
        )

        # combined[:, i, :] = [1.0 | dm | s (64) | mask] per row-chunk i
        comb = singles.tile([P, NCH, 67], f32)
        nc.vector.memset(comb[:, :, 0], 1.0)
        nc.vector.tensor_copy(out=comb[:, :, 66], in_=mask_sb)

        x_sb = singles.tile([P, NCH, F], f32)
        v_sb = singles.tile([P, NCH, C], f32)
        dsum = singles.tile([P, NCH], f32)
        stats_sb = singles.tile([C + 1, 66], f32)
        out_sb = singles.tile([C, F], f32)
        oa_sb = singles.tile([C, C], f32)
        q_sb = singles.tile([C, 6], f32)
        nc.vector.memset(q_sb[:, 5:6], 0.0)
        sc_sb = singles.tile([1, 12], f32)

        temps = ctx.enter_context(tc.tile_pool(name="temps", bufs=3))
        pans = ctx.enter_context(tc.tile_pool(name="pans", bufs=3))
        red = ctx.enter_context(tc.tile_pool(name="red", bufs=2))
        psA = ctx.enter_context(tc.tile_pool(name="psA", bufs=2, space="PSUM"))
        psAcc = ctx.enter_context(tc.tile_pool(name="psAcc", bufs=1, space="PSUM"))
        psV = ctx.enter_context(tc.tile_pool(name="psV", bufs=1, space="PSUM"))

        def emit_body():
            # persistent accumulators
            outT_ps = psAcc.tile([P, FCH, C], f32, tag="outT")

            # ---------- Phase A: s = softmax(x@W + b) * mask ----------
            for i in range(NCH):
                nc.sync.dma_start(out=x_sb[:, i, :], in_=x_r[:, i, :])
                lg_ps = psA.tile([P, C], f32, tag="lg")
                for fb in range(FCH):
                    xt_ps = psA.tile([P, P], f32, tag="xt")
                    nc.tensor.transpose(
                        xt_ps, x_sb[:, i, fb * P:(fb + 1) * P], ident)
                    xt_sb = temps.tile([P, P], f32, tag="xts")
                    nc.scalar.copy(out=xt_sb, in_=xt_ps)
                    nc.tensor.matmul(
                        lg_ps, xt_sb, W_sb[:, fb, :],
                        start=(fb == 0), stop=(fb == FCH - 1))
                lg_sb = temps.tile([P, C], f32, tag="lg_sb")
                nc.vector.tensor_add(out=lg_sb, in0=lg_ps, in1=b_bc)
                mx = temps.tile([P, 1], f32, tag="mx")
                nc.vector.reduce_max(mx, lg_sb, axis=AX.X)
                nc.vector.tensor_scalar_mul(mx, mx, -1.0)
                e_sb = temps.tile([P, C], f32, tag="e_sb")
                ssum = temps.tile([P, 1], f32, tag="ssum")
                nc.scalar.activation(out=e_sb, in_=lg_sb, func=AF.Exp,
                                     bias=mx, scale=1.0, accum_out=ssum)
                rs = temps.tile([P, 1], f32, tag="rs")
                nc.vector.reciprocal(rs, ssum)
                nc.vector.tensor_scalar(
                    out=comb[:, i, 2:66], in0=e_sb,
                    scalar1=rs, scalar2=mask_sb[:, i:i + 1],
                    op0=ALU.mult, op1=ALU.mult)
                nc.sync.dma_start(out=s_r[:, i, :], in_=comb[:, i, 2:66])
                # out^T accumulation: lhsT = x chunk block, rhs = s chunk.
                # start=True clears has_written for the whole bank, so only
                # the very first matmul into this bank starts the group.
                for fb in range(FCH):
                    nc.tensor.matmul(
                        outT_ps[:, fb, :], x_sb[:, i, fb * P:(fb + 1) * P],
                        comb[:, i, 2:66],
                        start=(i == 0 and fb == 0),
                        stop=(i == NCH - 1 and fb == FCH - 1))

            # ---------- adj panels: v = adj^T @ s, rowsums ----------
            for cb in range(NPAN):
                pan = pans.tile([P, NCH, PANW], f32, tag="pan")
                for og in range(4):
                    nc.sync.dma_start(
                        out=pan[:, og * 4:(og + 1) * 4, :],
                        in_=adj_r[:, og * 4:(og + 1) * 4,
                                  cb * PANW:(cb + 1) * PANW])
                v_ps = psV.tile([P, PBLK, C], f32, tag="vps")
                for j in range(PBLK):
                    for o in range(NCH):
                        nc.tensor.matmul(
                            v_ps[:, j, :],
                            pan[:, o, j * P:(j + 1) * P],
                            comb[:, o, 2:66],
                            start=(j == 0 and o == 0),
                            stop=(j == PBLK - 1 and o == NCH - 1))
                nc.vector.tensor_copy(
                    out=v_sb[:, cb * PBLK:(cb + 1) * PBLK, :], in_=v_ps)
                # partial row-sums of this panel
                rtmp = red.tile([P, NCH], f32, tag="rtmp")
                nc.vector.reduce_sum(rtmp, pan, axis=AX.X)
                if cb == 0:
                    nc.vector.tensor_copy(out=dsum, in_=rtmp)
                else:
                    nc.vector.tensor_add(out=dsum, in0=dsum, in1=rtmp)

            # dm = rowsum * mask -> comb col 1
            nc.vector.tensor_mul(out=comb[:, :, 1], in0=dsum, in1=mask_sb)

            # ---------- stats + out_adj accumulations ----------
            stats_ps = psAcc.tile([C + 1, 66], f32, tag="stats")
            oa_ps = psAcc.tile([C, C], f32, tag="oa")
            for i in range(NCH):
                nc.tensor.matmul(
                    stats_ps, comb[:, i, 2:67], comb[:, i, 0:66],
                    start=(i == 0), stop=(i == NCH - 1))
            for i in range(NCH):
                nc.tensor.matmul(
                    oa_ps, v_sb[:, i, :], comb[:, i, 2:66],
                    start=(i == 0), stop=(i == NCH - 1))
            nc.scalar.copy(out=stats_sb, in_=stats_ps)
            nc.scalar.copy(out=oa_sb, in_=oa_ps)

            # ---------- selu(out^T), transpose to out ----------
            neg = temps.tile([P, FCH, C], f32, tag="neg")
            nc.vector.tensor_scalar_min(neg, outT_ps, 0.0)
            e_t = temps.tile([P, FCH, C], f32, tag="e_t")
            nc.scalar.activation(out=e_t, in_=neg, func=AF.Exp)
            nc.vector.tensor_scalar(
                out=e_t, in0=e_t,
                scalar1=SELU_SCALE * SELU_ALPHA,
                scalar2=SELU_SCALE * SELU_ALPHA,
                op0=ALU.mult, op1=ALU.subtract)
            r_t = temps.tile([P, FCH, C], f32, tag="r_t")
            nc.scalar.activation(out=r_t, in_=outT_ps, func=AF.Relu,
                                 scale=SELU_SCALE)
            oT_sb = temps.tile([P, FCH, C], f32, tag="oT_sb")
            nc.vector.tensor_add(out=oT_sb, in0=r_t, in1=e_t)
            op_ps = psA.tile([C, F], f32, tag="lg")
            for fb in range(FCH):
                nc.tensor.transpose(
                    op_ps[:, fb * P:(fb + 1) * P], oT_sb[:, fb, :], ident)
            nc.scalar.copy(out=out_sb, in_=op_ps)
            nc.sync.dma_start(out=o_out[:], in_=out_sb)

            # ---------- scalar reductions ----------
            I64 = ident[:C, :C]
            dt1 = temps.tile([C, C], f32, tag="dt1")
            nc.vector.tensor_mul(out=dt1, in0=oa_sb, in1=I64)
            nc.vector.reduce_sum(q_sb[:, 0:1], dt1, axis=AX.X)
            dt2 = temps.tile([C, C], f32, tag="dt2")
            nc.vector.tensor_mul(out=dt2, in0=stats_sb[0:C, 2:66], in1=I64)
            nc.vector.reduce_sum(q_sb[:, 1:2], dt2, axis=AX.X)
            sqt = temps.tile([C, C], f32, tag="sqt")
            nc.scalar.activation(out=sqt, in_=stats_sb[0:C, 2:66],
                                 func=AF.Square, accum_out=q_sb[:, 2:3])
            nc.vector.tensor_mul(out=q_sb[:, 3:4], in0=stats_sb[0:C, 0:1],
                                 in1=stats_sb[0:C, 0:1])
            nc.vector.tensor_mul(out=q_sb[:, 4:5], in0=stats_sb[0:C, 1:2],
                                 in1=stats_sb[0:C, 1:2])
            sums_ps = psA.tile([1, 6], f32, tag="xt")
            nc.tensor.matmul(sums_ps, ones_col, q_sb, start=True, stop=True)
            sums_sb = temps.tile([1, 6], f32, tag="sums_sb")
            nc.vector.tensor_copy(out=sums_sb, in_=sums_ps)
            # sums_sb: [tr_oa, tr_ss, ssq, csq, caq, _]

            cm_sb = temps.tile([1, 2], f32, tag="cm_sb")  # [count, 2m]
            nc.sync.dma_start(out=cm_sb, in_=stats_sb[C:C + 1, 0:2])

            r2m = temps.tile([1, 1], f32, tag="r2m")
            nc.vector.reciprocal(r2m, cm_sb[:, 1:2])
            t1 = temps.tile([1, 1], f32, tag="t1")
            nc.vector.tensor_mul(out=t1, in0=sums_sb[:, 4:5], in1=r2m)
            nc.vector.tensor_sub(out=t1, in0=t1, in1=sums_sb[:, 0:1])
            nc.vector.tensor_mul(out=sc_sb[:, 0:1], in0=t1, in1=r2m)

            sq1 = temps.tile([1, 1], f32, tag="sq1")
            nc.scalar.activation(out=sq1, in_=sums_sb[:, 2:3], func=AF.Sqrt)
            nc.vector.reciprocal(sq1, sq1)
            nc.vector.tensor_mul(out=sq1, in0=sums_sb[:, 1:2], in1=sq1)
            nc.vector.tensor_scalar(out=sq1, in0=sq1, scalar1=-0.25,
                                    scalar2=2.0, op0=ALU.mult, op1=ALU.add)
            nc.scalar.activation(out=sc_sb[:, 1:2], in_=sq1, func=AF.Sqrt)
            nc.vector.tensor_copy(out=sc_sb[:, 2:3], in_=sums_sb[:, 3:4])
            nc.vector.tensor_copy(out=sc_sb[:, 3:4], in_=cm_sb[:, 0:1])
            nc.sync.dma_start(out=sc_out[:].rearrange("(p f) -> p f", p=1),
                              in_=sc_sb[:, 0:4])
            nc.sync.dma_start(out=cs_out[:].rearrange("(p f) -> p f", p=C),
                              in_=stats_sb[0:C, 0:1])

            # ---------- out_adj normalization ----------
            nc.vector.tensor_mul(out=oa_sb, in0=oa_sb, in1=invI)
            rsu = temps.tile([C, 1], f32, tag="rsu")
            nc.vector.reduce_sum(rsu, oa_sb, axis=AX.X)
            nc.scalar.activation(out=rsu, in_=rsu, func=AF.Sqrt)
            nc.vector.tensor_scalar_add(rsu, rsu, EPS)
            nc.vector.reciprocal(rsu, rsu)
            rdT_ps = psA.tile([1, C], f32, tag="lg")
            nc.tensor.matmul(rdT_ps, rsu, I64, start=True, stop=True)
            rdT_sb = temps.tile([1, C], f32, tag="rdT_sb")
            nc.vector.tensor_copy(out=rdT_sb, in_=rdT_ps)
            rr_ps = psA.tile([C, C], f32, tag="xt")
            nc.tensor.matmul(rr_ps, rdT_sb, rdT_sb, start=True, stop=True)
            nc.vector.tensor_mul(out=oa_sb, in0=oa_sb, in1=rr_ps)
            nc.sync.dma_start(out=oa_out[:], in_=oa_sb)

        if loop_n is None:
            emit_body()
        else:
            with tc.For_i(0, loop_n, 1):
                emit_body()

    nc.compile()
    return nc


_CACHE = {}


def _get_nc():
    if "nc" not in _CACHE:
        _CACHE["nc"] = build_program()
    return _CACHE["nc"]


def kernel(x, adj, mask, W, b):
    from concourse.bass_utils import run_bass_kernel_spmd

    x = np.asarray(x, np.float32)
    adj = np.asarray(adj, np.float32)
    maskf = np.asarray(mask).astype(np.float32)
    Wn = np.asarray(W, np.float32)
    bn = np.asarray(b, np.float32)
    B = x.shape[0]

    nc = _get_nc()
    in_maps = [
        {"x": x[i], "adj": adj[i], "maskf": maskf[i], "w": Wn, "bvec": bn}
        for i in range(B)
    ]
    res = run_bass_kernel_spmd(nc, in_maps, core_ids=list(range(B)))
    rr = res.results

    s = np.stack([rr[i]["s_out"] for i in range(B)])
    out = np.stack([rr[i]["o_out"] for i in range(B)])
    oa = np.stack([rr[i]["oa_out"] for i in range(B)])
    sc = np.stack([rr[i]["sc_out"] for i in range(B)])  # [B, 4]
    cs = np.stack([rr[i]["cs_out"] for i in range(B)])  # [B, C] f32

    spectral = np.float32(np.mean(sc[:, 0].astype(np.float32)))
    ortho = np.float32(np.mean(sc[:, 1].astype(np.float32)))
    # replicate the reference's f32 arithmetic: norm(cluster_size) / count
    # broadcast to [B, B], * sqrt(C), - 1, mean.
    norms = np.sqrt((cs * cs).sum(axis=1)).astype(np.float32)   # [B]
    counts = sc[:, 3].astype(np.float32)                        # [B]
    cl_mat = (norms[None, :] / counts[:, None]).astype(np.float32) \
        * np.float32(np.sqrt(np.float32(C))) - np.float32(1.0)
    cluster = np.float32(np.mean(cl_mat.astype(np.float32)))
    return s, out, oa, spectral, ortho, cluster
